# revision 1
# baseline (speedup 1.0000x reference)
"""Trainium2 Bass kernel for nn_AttentionModel (sparse banded attention).

Math (per batch element, data-parallel over 8 cores):
  qs    = q @ W_score.T
  score = qs @ k.T                      # only the 129-wide causal band matters
  w     = banded_softmax(score)         # full-row max cancels mathematically
  c     = w @ k
  enh   = tanh(concat([c, q]) @ W_enh.T + b_enh)
  out   = sigmoid(enh @ W_mask.T + b_mask)

Implementation notes:
  - T=2000 padded: keys get 128 zero rows in front + 48 tail -> 2176 = 17*128;
    queries get 48 tail pad -> 2048 = 16*128.  Query tile j attends key blocks
    j (prev) and j+1 (diag) of the padded key array.
  - Band masking is done by accumulating an additive -32768 mask into the score
    PSUM tile with an identity-weight matmul (PE is cheaper than DVE here).
  - Softmax skips the max subtraction (it cancels exactly; in-band |score|<~60
    so exp() is safe in fp32).  exp runs on ACT with accum_out giving the row
    sums for free; normalization is a per-partition tensor_scalar multiply.
  - sigmoid(x) is computed as 0.5*tanh(0.5x)+0.5 so the whole kernel uses the
    single "exp_and_others" ACT table set (exp+tanh) -> no table reloads.
  - Matmuls with moving free dim >= 256 use the float32r dtype (single-pass
    fp32 streaming, 1 cycle/row vs 4 for plain fp32 on TRN2).
  - The PV stage (c = w @ k) processes query-tile PAIRS so its moving operand
    is 256 wide: key block b multiplies the transposed-weight halves of both
    adjacent query tiles in one matmul.
"""

import sys
import types

import numpy as np
from contextlib import ExitStack

import concourse.bass as bass
import concourse.bacc as bacc
import concourse.tile as tile
from concourse import mybir
from concourse.bass_utils import run_bass_kernel_spmd


def _ensure_axon_hooks():
    # bass_utils imports antenv.axon_hooks when tracing is requested; some
    # images lack that module.  Register a shim built from the boot helper
    # so a BASS_TRACE=1 environment doesn't crash the kernel.
    try:
        from antenv import axon_hooks  # noqa: F401
        return
    except ImportError:
        pass
    try:
        from trn_agent_boot.trn_boot import _ntff_profile_via_ctypes
        hook = _ntff_profile_via_ctypes("/opt/axon/libaxon_pjrt.so")
    except Exception:
        hook = None
    m = types.ModuleType("antenv.axon_hooks")
    m.get_axon_ntff_profile_hook = lambda: hook
    m.set_axon_ntff_profile_hook = lambda h: None
    sys.modules["antenv.axon_hooks"] = m


_ensure_axon_hooks()

F32 = mybir.dt.float32
F32R = mybir.dt.float32r
AF = mybir.ActivationFunctionType
ALU = mybir.AluOpType

B, T, H, F_OUT = 8, 2000, 256, 257
TPK = 2176   # padded key length   (128 front + 2000 + 48 tail)
TPQ = 2048   # padded query length (2000 + 48 tail)
NT = 16      # query tiles of 128
NKB = 17     # key blocks of 128
NEG = -32768.0
OPAD = 258  # F_OUT padded even for fp32r ISA restrictions
N_CORES = 8

_CACHE = {}


def _consts():
    t_i = np.arange(128, dtype=np.int32)[:, None]
    s_i = np.arange(128, dtype=np.int32)[None, :]
    mask_prev = np.where(s_i >= t_i, 0.0, NEG).astype(np.float32)
    mask_diag = np.where(s_i <= t_i, 0.0, NEG).astype(np.float32)
    mask_std = np.ascontiguousarray(np.concatenate([mask_prev, mask_diag], 1))
    mask_t0 = np.ascontiguousarray(
        np.concatenate([np.full((128, 128), NEG, np.float32), mask_diag], 1)
    )
    ident = np.eye(128, dtype=np.float32)
    ones_row = np.ones((1, 128), dtype=np.float32)
    return ident, mask_std, mask_t0, ones_row


def build_nc():
    nc = bacc.Bacc("TRN2", target_bir_lowering=False, debug=False,
                   num_devices=N_CORES)

    kT = nc.declare_dram_parameter("kT", [H, TPK], F32R, isOutput=False)
    kN = nc.declare_dram_parameter("kN", [TPK, H], F32R, isOutput=False)
    qT = nc.declare_dram_parameter("qT", [H, TPQ], F32R, isOutput=False)
    WsT = nc.declare_dram_parameter("WsT", [H, H], F32R, isOutput=False)
    WeT = nc.declare_dram_parameter("WeT", [2 * H, H], F32R, isOutput=False)
    WmT = nc.declare_dram_parameter("WmT", [H, OPAD], F32R, isOutput=False)
    be = nc.declare_dram_parameter("be", [H, 1], F32, isOutput=False)
    bm = nc.declare_dram_parameter("bm", [128, OPAD], F32, isOutput=False)
    out = nc.declare_dram_parameter("out", [T, F_OUT], F32, isOutput=True)

    ident_np, mask_std_np, mask_t0_np, ones_np = _consts()
    ident_d = nc.inline_tensor(ident_np, "identc")
    mask_std_d = nc.inline_tensor(mask_std_np, "mask_stdc")
    mask_t0_d = nc.inline_tensor(mask_t0_np, "mask_t0c")
    ones_d = nc.inline_tensor(ones_np, "onesc")

    with tile.TileContext(nc) as tc, ExitStack() as ctx:
        const = ctx.enter_context(tc.tile_pool(name="const", bufs=1))
        io = ctx.enter_context(tc.tile_pool(name="io", bufs=1))
        wk = ctx.enter_context(tc.tile_pool(name="wk", bufs=6))
        stat = ctx.enter_context(tc.tile_pool(name="stat", bufs=8))
        pmm = ctx.enter_context(tc.tile_pool(name="pmm", bufs=2, space="PSUM"))
        psc = ctx.enter_context(tc.tile_pool(name="psc", bufs=2, space="PSUM"))
        pwt = ctx.enter_context(tc.tile_pool(name="pwt", bufs=2, space="PSUM"))
        pct = ctx.enter_context(tc.tile_pool(name="pct", bufs=2, space="PSUM"))

        def cload(tag, shape, src, dt=F32R):
            t = const.tile(shape, dt, tag=tag, name=tag)
            nc.sync.dma_start(t[:], src)
            return t

        # critical-path consts first: P0 needs only wst (+ ident for P1 mask)
        wst = [cload(f"wst{c}", [128, H], WsT[c * 128:(c + 1) * 128, :])
               for c in range(2)]
        ident = cload("ident", [128, 128], ident_d[:].bitcast(F32R))

        # ---- big persistent SBUF buffers ----
        # Loads are chunked in consumption order so compute starts as soon as
        # the first tiles' data lands instead of waiting for whole tensors.
        qT_t = [io.tile([128, TPQ], F32R, tag=f"qT{c}", name=f"qT{c}")
                for c in range(2)]
        for nb in range(4):
            for c in range(2):
                nc.sync.dma_start(
                    qT_t[c][:, nb * 512:(nb + 1) * 512],
                    qT[c * 128:(c + 1) * 128, nb * 512:(nb + 1) * 512])
        kT_t = [io.tile([128, TPK], F32R, tag=f"kT{c}", name=f"kT{c}")
                for c in range(2)]
        kN_t = io.tile([128, NKB * 256], F32R, tag="kN", name="kN_t")
        # 4 column chunks of kT (sync queue) interleaved with 4-5 block
        # groups of kN (gpsimd queue)
        kn_groups = [(0, 5), (5, 9), (9, 13), (13, 17)]
        for i in range(4):
            for c in range(2):
                nc.sync.dma_start(
                    kT_t[c][:, i * 544:(i + 1) * 544],
                    kT[c * 128:(c + 1) * 128, i * 544:(i + 1) * 544])
            b0, b1 = kn_groups[i]
            nc.gpsimd.dma_start(
                kN_t[:, b0 * 256: b1 * 256].rearrange(
                    "p (b h) -> p b h", h=256),
                kN[b0 * 128: b1 * 128, :].rearrange(
                    "(b p) h -> p b h", p=128))

        # remaining consts (used by P1-mask / P2 / P3, all later)
        mask_std = cload("mask_std", [128, 256], mask_std_d[:].bitcast(F32R))
        mask_t0 = cload("mask_t0", [128, 256], mask_t0_d[:].bitcast(F32R))
        bm_t = cload("bm", [128, OPAD], bm[:], dt=F32)
        wet = [cload(f"wet{d}", [128, H], WeT[d * 128:(d + 1) * 128, :])
               for d in range(4)]
        wmt = [cload(f"wmt{f}", [128, OPAD], WmT[f * 128:(f + 1) * 128, :])
               for f in range(2)]
        bet = [cload(f"bet{f}", [128, 1], be[f * 128:(f + 1) * 128, :], dt=F32)
               for f in range(2)]
        qsT_t = [io.tile([128, TPQ], F32R, tag=f"qsT{c}", name=f"qsT{c}")
                 for c in range(2)]
        cT_t = [io.tile([128, TPQ], F32R, tag=f"cT{c}", name=f"cT{c}")
                for c in range(2)]
        enhT_t = [io.tile([128, TPQ], F32R, tag=f"enhT{c}", name=f"enhT{c}")
                  for c in range(2)]

        # ---- P0: qsT[g, t] = (q @ W_score.T).T ----
        for c in range(2):          # g chunk (psum partition dim)
            for nb in range(4):     # 512-wide t' blocks
                ps = pmm.tile([128, 512], F32, tag="mm", name="ps")
                for h in range(2):  # contraction chunk
                    nc.tensor.matmul(
                        ps[:],
                        wst[h][:, c * 128:(c + 1) * 128],
                        qT_t[h][:, nb * 512:(nb + 1) * 512],
                        start=(h == 0), stop=(h == 1))
                nc.vector.tensor_copy(qsT_t[c][:, nb * 512:(nb + 1) * 512], ps[:])

        # ---- P1 per query tile: scores -> softmax -> transposed weights ----
        def p1(j, wTP, l):
            # scores[t', s-window 256] + additive band mask, via PSUM accum
            ps = psc.tile([128, 256], F32, tag="sc", name="ps")
            for c in range(2):
                nc.tensor.matmul(
                    ps[:],
                    qsT_t[c][:, j * 128:(j + 1) * 128],
                    kT_t[c][:, j * 128: j * 128 + 256],
                    start=(c == 0), stop=False)
            nc.tensor.matmul(ps[:], ident[:],
                             (mask_t0 if j == 0 else mask_std)[:],
                             start=False, stop=True)
            # exp (no max subtraction needed) + row sums
            e_t = wk.tile([128, 256], F32, tag="e", name="e_t")
            den = stat.tile([128, 1], F32, tag="den", name="den")
            nc.scalar.activation(e_t[:], ps[:], AF.Exp, accum_out=den[:])
            rec = stat.tile([128, 1], F32, tag="rec", name="rec")
            nc.vector.reciprocal(rec[:], den[:])
            w_t = wk.tile([128, 256], F32R, tag="w", name="w_t")
            nc.vector.tensor_scalar_mul(w_t[:], e_t[:], rec[:])
            # transpose w -> [s', t'], scatter halves into the pair buffer:
            # wTP column layout is [block m=0 | m=1 | m=2] x 256 cols each,
            # within block m the 128-col half l is query tile 2p+l.
            pw = pwt.tile([128, 256], F32R, tag="pw", name="pw")
            nc.tensor.transpose(pw[:, 0:128], w_t[:, 0:128], ident[:])
            nc.tensor.transpose(pw[:, 128:256], w_t[:, 128:256], ident[:])
            # prev block is m=l, diag block is m=l+1: two 128-col regions
            # 256 cols apart -> one strided copy
            dst = wTP[:, l * 384: l * 384 + 384].rearrange(
                "p (m f) -> p m f", f=128)[:, 0:3:2, :]
            nc.vector.tensor_copy(dst, pw[:].rearrange("p (m f) -> p m f", f=128))

        # ---- pair PV: cT[h, t'pair] = sum over 3 key blocks ----
        def pair_pv(p):
            wTP = _CACHE["wTP_cur"]
            pc = pct.tile([128, 512], F32, tag="pc", name="pc")
            for h in range(2):
                for m in range(3):
                    blk = 2 * p + m
                    nc.tensor.matmul(
                        pc[:, h * 256:(h + 1) * 256],
                        kN_t[:, blk * 256 + h * 128: blk * 256 + (h + 1) * 128],
                        wTP[:, m * 256:(m + 1) * 256],
                        start=(m == 0), stop=(m == 2))
            for h in range(2):
                nc.scalar.copy(
                    cT_t[h][:, 2 * p * 128: 2 * p * 128 + 256],
                    pc[:, h * 256:(h + 1) * 256])

        def p2(nb):
            # enhT[f, t'] = tanh(W_enh.T stacked over [cT, qT] + b_enh)
            rhs_tiles = [cT_t[0], cT_t[1], qT_t[0], qT_t[1]]
            for f in range(2):
                pe_ = pmm.tile([128, 512], F32, tag="mm", name="pe_")
                for d in range(4):
                    nc.tensor.matmul(
                        pe_[:],
                        wet[d][:, f * 128:(f + 1) * 128],
                        rhs_tiles[d][:, nb * 512:(nb + 1) * 512],
                        start=(d == 0), stop=(d == 3))
                nc.scalar.activation(enhT_t[f][:, nb * 512:(nb + 1) * 512],
                                     pe_[:], AF.Tanh, bias=bet[f][:, 0:1])

        def p3(j):
            # z = enh @ W_mask.T + b_mask ; out = sigmoid(z) = 0.5*tanh(z/2)+0.5
            pm = pmm.tile([128, OPAD], F32, tag="mm", name="pm")
            for f in range(2):
                nc.tensor.matmul(pm[:],
                                 enhT_t[f][:, j * 128:(j + 1) * 128],
                                 wmt[f][:], start=(f == 0), stop=(f == 1))
            z_t = wk.tile([128, OPAD], F32, tag="z", name="z_t")
            nc.vector.tensor_add(z_t[:], pm[:], bm_t[:])
            o_t = wk.tile([128, OPAD], F32, tag="o", name="o_t")
            nc.scalar.activation(o_t[:], z_t[:], AF.Tanh, scale=0.5)
            o2_t = wk.tile([128, OPAD], F32, tag="o2", name="o2_t")
            nc.gpsimd.tensor_scalar(o2_t[:], o_t[:], 0.5, 0.5,
                                    op0=ALU.mult, op1=ALU.add)
            rows = min(128, T - j * 128)
            nc.sync.dma_start(out[j * 128: j * 128 + rows, :], o2_t[0:rows, 0:F_OUT])

        for p in range(NT // 2):
            wTP = wk.tile([128, 768], F32R, tag="wTP", name="wTP")
            _CACHE["wTP_cur"] = wTP
            # boundary halves never written by transposes -> zero them
            nc.gpsimd.memset(wTP[:, 128:256].bitcast(F32), 0.0)
            nc.gpsimd.memset(wTP[:, 512:640].bitcast(F32), 0.0)
            p1(2 * p, wTP, 0)
            p1(2 * p + 1, wTP, 1)
            pair_pv(p)
            if p % 2 == 1:
                nb = p // 2
                p2(nb)
                for jj in range(nb * 4, nb * 4 + 4):
                    p3(jj)
        _CACHE.pop("wTP_cur", None)

    return nc


def _prep_shared(W_score, W_enh, b_enh, W_mask, b_mask):
    WsT = np.ascontiguousarray(W_score.T.astype(np.float32))        # [h, g]
    WeT = np.ascontiguousarray(W_enh.T.astype(np.float32))          # [d, f]
    WmT = np.zeros((H, 258), np.float32)                            # [f, o+pad]
    WmT[:, :F_OUT] = W_mask.T.astype(np.float32)
    be = np.ascontiguousarray(b_enh.astype(np.float32).reshape(H, 1))
    bm = np.zeros((128, 258), np.float32)
    bm[:, :F_OUT] = b_mask.astype(np.float32)[None, :]
    return WsT, WeT, WmT, be, bm


def make_in_maps(k, q, W_score, W_enh, b_enh, W_mask, b_mask):
    k = np.asarray(k, np.float32)
    q = np.asarray(q, np.float32)
    WsT, WeT, WmT, be, bm = _prep_shared(
        np.asarray(W_score, np.float32), np.asarray(W_enh, np.float32),
        np.asarray(b_enh, np.float32), np.asarray(W_mask, np.float32),
        np.asarray(b_mask, np.float32))
    in_maps = []
    for b in range(N_CORES):
        kb = np.zeros((TPK, H), np.float32)
        kb[128:128 + T] = k[b]
        qb = np.zeros((TPQ, H), np.float32)
        qb[:T] = q[b]
        in_maps.append({
            "kT": np.ascontiguousarray(kb.T),
            "kN": kb,
            "qT": np.ascontiguousarray(qb.T),
            "WsT": WsT, "WeT": WeT, "WmT": WmT, "be": be, "bm": bm,
        })
    return in_maps


def get_nc():
    if "nc" not in _CACHE:
        nc = build_nc()
        nc.finalize()
        _CACHE["nc"] = nc
    return _CACHE["nc"]


def kernel(k, q, W_score, W_enh, b_enh, W_mask, b_mask):
    in_maps = make_in_maps(k, q, W_score, W_enh, b_enh, W_mask, b_mask)
    res = run_bass_kernel_spmd(get_nc(), in_maps, list(range(N_CORES)))
    return np.stack([r["out"] for r in res.results], 0)



# revision 5
# speedup vs baseline: 1.1949x; 1.1949x over previous
"""Trainium2 Bass kernel for nn_AttentionModel (sparse banded attention).

Math (per batch element, data-parallel over 8 cores):
  qs    = q @ W_score.T
  score = qs @ k.T                      # only the 129-wide causal band matters
  w     = banded_softmax(score)         # full-row max cancels mathematically
  c     = w @ k
  enh   = tanh(concat([c, q]) @ W_enh.T + b_enh)
  out   = sigmoid(enh @ W_mask.T + b_mask)

Implementation (v2, transposed-score structure):
  - T=2000 padded to 2048 on both query and key axes (16 blocks of 128).
    No front padding: key block m holds scores for query tiles j=m (diagonal
    relation, keep s'<=t') and j=m+1 (previous-block relation, keep s'>=t').
  - Scores are computed TRANSPOSED per key block: psum[s',t'2tiles] =
    kT_blk^T @ qsT window.  Band masking is a DVE tensor_tensor add of a
    single constant [diag|prev] mask; exp runs on ACT writing bf16 w~T tiles
    that are already in the layout PV wants (no w transposes).
  - PV appends a ones column to k: c~[t',258] = w~T.T @ [kN|1], so column 256
    is the softmax denominator for free.  Normalization is one reciprocal +
    one per-partition tensor_scalar multiply per tile.  c is then transposed
    (PE, bf16, 1 cyc/row) into feature-major cT for the enhancement matmul.
  - Final stage is computed TRANSPOSED (outT[o,t']) so b_mask rides the ACT
    per-partition bias port, sigmoid(x)=0.5*tanh(0.5x)+0.5 uses the same
    exp_and_others table set, and the output DMA gets 1KB+ rows (bf16).
    The host un-transposes and upcasts.
  - dtypes: the softmax-sensitive path (q,k,W_score,qs) stays fp32r (same
    1 cycle/row as bf16 on the PE); everything downstream is bf16
    (measured end-to-end rel err 3.8e-3 vs 2e-2 tolerance).
  - PE stream is software-pipelined: scores run 2 blocks ahead of PV,
    3 ahead of the c transposes, with P2/P3 groups interleaved, so the
    DVE-mask -> ACT-exp chain never stalls the tensor engine.
"""

import sys
import types

import numpy as np
import ml_dtypes
from contextlib import ExitStack

import concourse.bass as bass
import concourse.bacc as bacc
import concourse.tile as tile
from concourse import mybir
from concourse.bass_utils import run_bass_kernel_spmd


def _ensure_axon_hooks():
    try:
        from antenv import axon_hooks  # noqa: F401
        return
    except ImportError:
        pass
    try:
        from trn_agent_boot.trn_boot import _ntff_profile_via_ctypes
        hook = _ntff_profile_via_ctypes("/opt/axon/libaxon_pjrt.so")
    except Exception:
        hook = None
    m = types.ModuleType("antenv.axon_hooks")
    m.get_axon_ntff_profile_hook = lambda: hook
    m.set_axon_ntff_profile_hook = lambda h: None
    sys.modules["antenv.axon_hooks"] = m


_ensure_axon_hooks()

F32 = mybir.dt.float32
F32R = mybir.dt.float32r
BF16 = mybir.dt.bfloat16
AF = mybir.ActivationFunctionType
ALU = mybir.AluOpType

B, T, H, F_OUT = 8, 2000, 256, 257
TQ = 2048          # padded query/key length (16 tiles of 128)
NT = 16            # tiles/blocks of 128
KW = 258           # kN row width: 256 features + ones col + pad col
NEG = -30000.0
N_CORES = 8

_CACHE = {}


def build_nc():
    nc = bacc.Bacc("TRN2", target_bir_lowering=False, debug=False,
                   num_devices=N_CORES)

    qT = nc.declare_dram_parameter("qT", [H, TQ], F32R, isOutput=False)
    kT = nc.declare_dram_parameter("kT", [H, TQ], F32R, isOutput=False)
    kN = nc.declare_dram_parameter("kN", [TQ, KW], BF16, isOutput=False)
    WsT = nc.declare_dram_parameter("WsT", [H, H], F32R, isOutput=False)
    WeTq = nc.declare_dram_parameter("WeTq", [H, H], F32R, isOutput=False)
    blobF = nc.declare_dram_parameter("blobF", [128, 261], F32, isOutput=False)
    blobB = nc.declare_dram_parameter("blobB", [128, 1408], BF16,
                                      isOutput=False)
    out = nc.declare_dram_parameter("out", [KW, TQ], BF16, isOutput=True)

    with tile.TileContext(nc) as tc, ExitStack() as ctx:
        const = ctx.enter_context(tc.tile_pool(name="const", bufs=1))
        io = ctx.enter_context(tc.tile_pool(name="io", bufs=1))
        wk = ctx.enter_context(tc.tile_pool(name="wk", bufs=1))
        stat = ctx.enter_context(tc.tile_pool(name="stat", bufs=1))
        pmm = ctx.enter_context(tc.tile_pool(name="pmm", bufs=2, space="PSUM"))
        psc = ctx.enter_context(tc.tile_pool(name="psc", bufs=2, space="PSUM"))
        ppv = ctx.enter_context(tc.tile_pool(name="ppv", bufs=2, space="PSUM"))
        ptr = ctx.enter_context(tc.tile_pool(name="ptr", bufs=2, space="PSUM"))

        # ---- persistent SBUF ----
        wst = [const.tile([128, H], F32R, tag=f"wst{c}", name=f"wst{c}")
               for c in range(2)]
        weq = [const.tile([128, H], F32R, tag=f"weq{c}", name=f"weq{c}")
               for c in range(2)]
        blobF_t = const.tile([128, 261], F32, tag="blobF", name="blobF_t")
        blobB_t = const.tile([128, 1408], BF16, tag="blobB", name="blobB_t")

        qT_t = [io.tile([128, TQ], F32R, tag=f"qT{c}", name=f"qT{c}")
                for c in range(2)]
        kT_t = [io.tile([128, TQ], F32R, tag=f"kT{c}", name=f"kT{c}")
                for c in range(2)]
        kN_t = io.tile([128, NT * KW], BF16, tag="kN", name="kN_t")
        qsT_t = [io.tile([128, TQ], F32R, tag=f"qsT{c}", name=f"qsT{c}")
                 for c in range(2)]
        cT_t = [io.tile([128, TQ], BF16, tag=f"cT{c}", name=f"cT{c}")
                for c in range(2)]
        enhT_t = [io.tile([128, TQ], BF16, tag=f"enhT{c}", name=f"enhT{c}")
                  for c in range(2)]
        oT_sb = [io.tile([128, TQ], BF16, tag=f"oT{c}", name=f"oT{c}")
                 for c in range(2)]
        oT_row = io.tile([1, TQ], BF16, tag="oTr", name="oT_row")

        # const views
        maskC = blobF_t[:, 0:256]              # [diag keep s'<=t' | prev s'>=t']
        beS = [blobF_t[:, 256 + f:257 + f] for f in range(2)]
        bmS = [blobF_t[:, 258 + ci:259 + ci] for ci in range(3)]
        identB = blobB_t[:, 0:128]
        web = [blobB_t[:, 128 + d * 256: 128 + (d + 1) * 256] for d in range(2)]
        wmp = [blobB_t[:, 640 + f * 384: 640 + (f + 1) * 384] for f in range(2)]

        # ---- DMA loads: sync queue in consumption-priority order ----
        nc.sync.dma_start(wst[0][:], WsT[0:128, :])
        nc.sync.dma_start(wst[1][:], WsT[128:256, :])
        nc.sync.dma_start(qT_t[0][:, 0:512], qT[0:128, 0:512])
        nc.sync.dma_start(qT_t[1][:, 0:512], qT[128:256, 0:512])
        nc.sync.dma_start(kT_t[0][:, 0:512], kT[0:128, 0:512])
        nc.sync.dma_start(kT_t[1][:, 0:512], kT[128:256, 0:512])
        nc.sync.dma_start(blobF_t[:], blobF[:])
        nc.sync.dma_start(qT_t[0][:, 512:1024], qT[0:128, 512:1024])
        nc.sync.dma_start(qT_t[1][:, 512:1024], qT[128:256, 512:1024])
        nc.sync.dma_start(blobB_t[:], blobB[:])
        nc.sync.dma_start(kT_t[0][:, 512:1024], kT[0:128, 512:1024])
        nc.sync.dma_start(kT_t[1][:, 512:1024], kT[128:256, 512:1024])
        nc.sync.dma_start(qT_t[0][:, 1024:1536], qT[0:128, 1024:1536])
        nc.sync.dma_start(qT_t[1][:, 1024:1536], qT[128:256, 1024:1536])
        nc.sync.dma_start(weq[0][:], WeTq[0:128, :])
        nc.sync.dma_start(weq[1][:], WeTq[128:256, :])
        nc.sync.dma_start(kT_t[0][:, 1024:1536], kT[0:128, 1024:1536])
        nc.sync.dma_start(kT_t[1][:, 1024:1536], kT[128:256, 1024:1536])
        nc.sync.dma_start(qT_t[0][:, 1536:2048], qT[0:128, 1536:2048])
        nc.sync.dma_start(qT_t[1][:, 1536:2048], qT[128:256, 1536:2048])
        nc.sync.dma_start(kT_t[0][:, 1536:2048], kT[0:128, 1536:2048])
        nc.sync.dma_start(kT_t[1][:, 1536:2048], kT[128:256, 1536:2048])
        # kN on the gpsimd queue (2 chunks, first 8 blocks needed early)
        for (b0, b1) in ((0, 8), (8, 16)):
            nc.gpsimd.dma_start(
                kN_t[:, b0 * KW: b1 * KW].rearrange("p (m h) -> p m h", h=KW),
                kN[b0 * 128: b1 * 128, :].rearrange("(m p) h -> p m h", p=128))

        # ---- stage emitters ----
        def emit_p0(nb):
            # qsT[g, t'] = (q @ W_score.T).T : stationary wst, moving qT
            for c in range(2):
                ps = pmm.tile([128, 512], F32, tag="mm", name="ps")
                for h in range(2):
                    nc.tensor.matmul(
                        ps[:],
                        wst[h][:, c * 128:(c + 1) * 128],
                        qT_t[h][:, nb * 512:(nb + 1) * 512],
                        start=(h == 0), stop=(h == 1))
                nc.vector.tensor_copy(qsT_t[c][:, nb * 512:(nb + 1) * 512],
                                      ps[:])

        def emit_sc(m):
            # scoreT[s' of block m, t' of tiles m,m+1] + mask -> exp -> w~T
            wcols = 256 if m < NT - 1 else 128
            ps = psc.tile([128, 256], F32, tag="sc", name="ps")
            for g in range(2):
                nc.tensor.matmul(
                    ps[:, 0:wcols],
                    kT_t[g][:, m * 128:(m + 1) * 128],
                    qsT_t[g][:, m * 128: m * 128 + wcols],
                    start=(g == 0), stop=(g == 1))
            sb = wk.tile([128, 256], F32, tag="scb", bufs=3, name="sb")
            nc.vector.tensor_add(sb[:, 0:wcols], ps[:, 0:wcols],
                                 maskC[:, 0:wcols])
            wt = wk.tile([128, 256], BF16, tag="wt", bufs=5, name="wt")
            nc.scalar.activation(wt[:, 0:wcols], sb[:, 0:wcols], AF.Exp)
            return wt

        wT = [None] * NT

        def emit_pv(j):
            # c~[t', 258] = sum_blocks w~T.T @ [kN | 1]
            pc = ppv.tile([128, KW], F32, tag="pv", name="pc")
            if j == 0:
                nc.tensor.matmul(pc[:], wT[0][:, 0:128],
                                 kN_t[:, 0:KW], start=True, stop=True)
            else:
                nc.tensor.matmul(pc[:], wT[j - 1][:, 128:256],
                                 kN_t[:, (j - 1) * KW: j * KW],
                                 start=True, stop=False)
                nc.tensor.matmul(pc[:], wT[j][:, 0:128],
                                 kN_t[:, j * KW: (j + 1) * KW],
                                 start=False, stop=True)
            rec = stat.tile([128, 1], F32, tag="rec", bufs=4, name="rec")
            nc.vector.reciprocal(rec[:], pc[:, 256:257])
            cb = wk.tile([128, 256], BF16, tag="cb", bufs=4, name="cb")
            nc.vector.tensor_scalar_mul(cb[:], pc[:, 0:256], rec[:])
            return cb

        cB = [None] * NT

        def emit_tr(j):
            # cT[h, t'] via PE transpose (bf16), gpsimd copies psum->sbuf
            for h in range(2):
                pt = ptr.tile([128, 128], BF16, tag="tr", name="pt")
                nc.tensor.transpose(pt[:], cB[j][:, h * 128:(h + 1) * 128],
                                    identB)
                # gpsimd cannot read PSUM; split copies across DVE and ACT
                if h == 0:
                    nc.vector.tensor_copy(cT_t[h][:, j * 128:(j + 1) * 128],
                                          pt[:])
                else:
                    nc.scalar.copy(cT_t[h][:, j * 128:(j + 1) * 128], pt[:])

        def emit_group(g):
            # P2: enhT[f, t'] = tanh(W_enh.T @ [cT;qT] + be)
            for f in range(2):
                pe_ = pmm.tile([128, 512], F32, tag="mm", name="pe_")
                nc.tensor.matmul(pe_[:], web[0][:, f * 128:(f + 1) * 128],
                                 cT_t[0][:, g * 512:(g + 1) * 512],
                                 start=True, stop=False)
                nc.tensor.matmul(pe_[:], web[1][:, f * 128:(f + 1) * 128],
                                 cT_t[1][:, g * 512:(g + 1) * 512],
                                 start=False, stop=False)
                nc.tensor.matmul(pe_[:], weq[0][:, f * 128:(f + 1) * 128],
                                 qT_t[0][:, g * 512:(g + 1) * 512],
                                 start=False, stop=False)
                nc.tensor.matmul(pe_[:], weq[1][:, f * 128:(f + 1) * 128],
                                 qT_t[1][:, g * 512:(g + 1) * 512],
                                 start=False, stop=True)
                nc.scalar.activation(enhT_t[f][:, g * 512:(g + 1) * 512],
                                     pe_[:], AF.Tanh, bias=beS[f])
            # P3 transposed: outT[o, t'] = 0.5*tanh(0.5*(Wm@enh + bm)) + 0.5
            for ci in range(3):
                p3 = pmm.tile([128, 512], F32, tag="mm", name="p3")
                for f in range(2):
                    nc.tensor.matmul(
                        p3[:], wmp[f][:, ci * 128:(ci + 1) * 128],
                        enhT_t[f][:, g * 512:(g + 1) * 512],
                        start=(f == 0), stop=(f == 1))
                if ci < 2:
                    os = wk.tile([128, 512], BF16, tag="os", bufs=2, name="os")
                    nc.scalar.activation(os[:], p3[:], AF.Tanh,
                                         scale=0.5, bias=bmS[ci])
                    nc.gpsimd.tensor_scalar(
                        oT_sb[ci][:, g * 512:(g + 1) * 512], os[:],
                        0.5, 0.5, op0=ALU.mult, op1=ALU.add)
                else:
                    os1 = wk.tile([1, 512], BF16, tag="os1", bufs=2,
                                  name="os1")
                    nc.scalar.activation(os1[:], p3[0:1, :], AF.Tanh,
                                         scale=0.5, bias=bmS[2][0:1, :])
                    nc.gpsimd.tensor_scalar(
                        oT_row[0:1, g * 512:(g + 1) * 512], os1[:],
                        0.5, 0.5, op0=ALU.mult, op1=ALU.add)

        def emit_stores(half):
            c0, c1 = half * 1024, (half + 1) * 1024
            nc.sync.dma_start(out[0:128, c0:c1], oT_sb[0][:, c0:c1])
            nc.sync.dma_start(out[128:256, c0:c1], oT_sb[1][:, c0:c1])
            nc.sync.dma_start(out[256:257, c0:c1], oT_row[0:1, c0:c1])

        # ---- software-pipelined emission ----
        LPV, LTR = 2, 3     # pv lags scores by 2 steps, transposes by 3

        def emit_lagged(step):
            jpv = step - LPV
            if 0 <= jpv < NT:
                cB[jpv] = emit_pv(jpv)
            jtr = step - LTR
            if 0 <= jtr < NT:
                emit_tr(jtr)
            if step >= 7 and (step - 7) % 4 == 0 and (step - 7) // 4 < 4:
                g = (step - 7) // 4
                emit_group(g)
                if g == 1:
                    emit_stores(0)
                elif g == 3:
                    emit_stores(1)

        m_next = 0
        for nb in range(4):
            emit_p0(nb)
            hi = 4 * nb + 2 if nb < 3 else NT - 1
            while m_next <= hi:
                wT[m_next] = emit_sc(m_next)
                emit_lagged(m_next)
                m_next += 1
        for step in range(NT, NT + 8):
            emit_lagged(step)

    return nc


def _pad_T(x):
    p = np.zeros((TQ, H), np.float32)
    p[:T] = x
    return p


def make_in_maps(k, q, W_score, W_enh, b_enh, W_mask, b_mask):
    k = np.asarray(k, np.float32)
    q = np.asarray(q, np.float32)
    W_score = np.asarray(W_score, np.float32)
    W_enh = np.asarray(W_enh, np.float32)
    b_enh = np.asarray(b_enh, np.float32)
    W_mask = np.asarray(W_mask, np.float32)
    b_mask = np.asarray(b_mask, np.float32)

    WsT = np.ascontiguousarray(W_score.T)                  # [h, g]
    WeT = W_enh.T                                          # [d, f] (512, 256)
    WeTq = np.ascontiguousarray(WeT[H:2 * H])              # q-feature half

    sI = np.arange(128, dtype=np.int32)[:, None]
    tI = np.arange(128, dtype=np.int32)[None, :]
    diag = np.where(sI <= tI, 0.0, NEG).astype(np.float32)
    prev = np.where(sI >= tI, 0.0, NEG).astype(np.float32)
    blobF = np.zeros((128, 261), np.float32)
    blobF[:, 0:128] = diag
    blobF[:, 128:256] = prev
    blobF[:, 256:258] = b_enh.reshape(2, 128).T
    bmh = np.zeros(384, np.float32)
    bmh[:F_OUT] = 0.5 * b_mask
    blobF[:, 258:261] = bmh.reshape(3, 128).T

    blobB = np.zeros((128, 1408), np.float32)
    blobB[:, 0:128] = np.eye(128, dtype=np.float32)
    blobB[:, 128:384] = WeT[0:128]
    blobB[:, 384:640] = WeT[128:256]
    WmP = np.zeros((H, 384), np.float32)
    WmP[:, :F_OUT] = W_mask.T
    blobB[:, 640:1024] = WmP[0:128]
    blobB[:, 1024:1408] = WmP[128:256]
    blobB = blobB.astype(ml_dtypes.bfloat16)

    in_maps = []
    for b in range(N_CORES):
        kb = _pad_T(k[b])
        qb = _pad_T(q[b])
        kNb = np.zeros((TQ, KW), np.float32)
        kNb[:, 0:H] = kb
        kNb[:, 256] = 1.0
        in_maps.append({
            "qT": np.ascontiguousarray(qb.T),
            "kT": np.ascontiguousarray(kb.T),
            "kN": kNb.astype(ml_dtypes.bfloat16),
            "WsT": WsT, "WeTq": WeTq,
            "blobF": blobF, "blobB": blobB,
        })
    return in_maps


def assemble_output(results):
    outs = []
    for r in results:
        o = np.asarray(r["out"]).astype(np.float32)        # [258, 2048]
        outs.append(np.ascontiguousarray(o[:F_OUT, :T].T))  # [2000, 257]
    return np.stack(outs, 0)


def get_nc():
    if "nc" not in _CACHE:
        nc = build_nc()
        nc.finalize()
        _CACHE["nc"] = nc
    return _CACHE["nc"]


def kernel(k, q, W_score, W_enh, b_enh, W_mask, b_mask):
    in_maps = make_in_maps(k, q, W_score, W_enh, b_enh, W_mask, b_mask)
    res = run_bass_kernel_spmd(get_nc(), in_maps, list(range(N_CORES)))
    return assemble_output(res.results)


# revision 6
# speedup vs baseline: 1.2740x; 1.0662x over previous
"""Trainium2 Bass kernel for nn_AttentionModel (sparse banded attention).

Math (per batch element, data-parallel over 8 cores):
  qs    = q @ W_score.T
  score = qs @ k.T                      # only the 129-wide causal band matters
  w     = banded_softmax(score)         # full-row max cancels mathematically
  c     = w @ k
  enh   = tanh(concat([c, q]) @ W_enh.T + b_enh)
  out   = sigmoid(enh @ W_mask.T + b_mask)

Implementation (v2, transposed-score structure):
  - T=2000 padded to 2048 on both query and key axes (16 blocks of 128).
    No front padding: key block m holds scores for query tiles j=m (diagonal
    relation, keep s'<=t') and j=m+1 (previous-block relation, keep s'>=t').
  - Scores are computed TRANSPOSED per key block: psum[s',t'2tiles] =
    kT_blk^T @ qsT window.  Band masking is a DVE tensor_tensor add of a
    single constant [diag|prev] mask; exp runs on ACT writing bf16 w~T tiles
    that are already in the layout PV wants (no w transposes).
  - PV appends a ones column to k: c~[t',258] = w~T.T @ [kN|1], so column 256
    is the softmax denominator for free.  Normalization is one reciprocal +
    one per-partition tensor_scalar multiply per tile.  c is then transposed
    (PE, bf16, 1 cyc/row) into feature-major cT for the enhancement matmul.
  - Final stage is computed TRANSPOSED (outT[o,t']) so b_mask rides the ACT
    per-partition bias port, sigmoid(x)=0.5*tanh(0.5x)+0.5 uses the same
    exp_and_others table set, and the output DMA gets 1KB+ rows (bf16).
    The host un-transposes and upcasts.
  - dtypes: the softmax-sensitive path (q,k,W_score,qs) stays fp32r (same
    1 cycle/row as bf16 on the PE); everything downstream is bf16
    (measured end-to-end rel err 3.8e-3 vs 2e-2 tolerance).
  - PE stream is software-pipelined: scores run 2 blocks ahead of PV,
    3 ahead of the c transposes, with P2/P3 groups interleaved, so the
    DVE-mask -> ACT-exp chain never stalls the tensor engine.
"""

import sys
import types

import numpy as np
import ml_dtypes
from contextlib import ExitStack

import concourse.bass as bass
import concourse.bacc as bacc
import concourse.tile as tile
from concourse import mybir
from concourse.bass_utils import run_bass_kernel_spmd


def _ensure_axon_hooks():
    try:
        from antenv import axon_hooks  # noqa: F401
        return
    except ImportError:
        pass
    try:
        from trn_agent_boot.trn_boot import _ntff_profile_via_ctypes
        hook = _ntff_profile_via_ctypes("/opt/axon/libaxon_pjrt.so")
    except Exception:
        hook = None
    m = types.ModuleType("antenv.axon_hooks")
    m.get_axon_ntff_profile_hook = lambda: hook
    m.set_axon_ntff_profile_hook = lambda h: None
    sys.modules["antenv.axon_hooks"] = m


_ensure_axon_hooks()

F32 = mybir.dt.float32
F32R = mybir.dt.float32r
BF16 = mybir.dt.bfloat16
AF = mybir.ActivationFunctionType
ALU = mybir.AluOpType

B, T, H, F_OUT = 8, 2000, 256, 257
TQ = 2048          # padded query/key length (16 tiles of 128)
NT = 16            # tiles/blocks of 128
KW = 258           # kN row width: 256 features + ones col + pad col
NEG = -30000.0
N_CORES = 8

_CACHE = {}


def build_nc():
    nc = bacc.Bacc("TRN2", target_bir_lowering=False, debug=False,
                   num_devices=N_CORES)

    qT = nc.declare_dram_parameter("qT", [H, TQ], F32R, isOutput=False)
    kT = nc.declare_dram_parameter("kT", [H, TQ], F32R, isOutput=False)
    kN = nc.declare_dram_parameter("kN", [128, NT * KW], BF16,
                                   isOutput=False)
    WsT = nc.declare_dram_parameter("WsT", [H, H], F32R, isOutput=False)
    WeTq = nc.declare_dram_parameter("WeTq", [H, H], F32R, isOutput=False)
    blobF = nc.declare_dram_parameter("blobF", [128, 261], F32, isOutput=False)
    blobB = nc.declare_dram_parameter("blobB", [128, 1408], BF16,
                                      isOutput=False)
    out = nc.declare_dram_parameter("out", [KW, TQ], BF16, isOutput=True)

    with tile.TileContext(nc) as tc, ExitStack() as ctx:
        const = ctx.enter_context(tc.tile_pool(name="const", bufs=1))
        io = ctx.enter_context(tc.tile_pool(name="io", bufs=1))
        wk = ctx.enter_context(tc.tile_pool(name="wk", bufs=1))
        stat = ctx.enter_context(tc.tile_pool(name="stat", bufs=1))
        pmm = ctx.enter_context(tc.tile_pool(name="pmm", bufs=2, space="PSUM"))
        psc = ctx.enter_context(tc.tile_pool(name="psc", bufs=2, space="PSUM"))
        ppv = ctx.enter_context(tc.tile_pool(name="ppv", bufs=2, space="PSUM"))
        ptr = ctx.enter_context(tc.tile_pool(name="ptr", bufs=2, space="PSUM"))

        # ---- persistent SBUF ----
        wst = [const.tile([128, H], F32R, tag=f"wst{c}", name=f"wst{c}")
               for c in range(2)]
        weq = [const.tile([128, H], F32R, tag=f"weq{c}", name=f"weq{c}")
               for c in range(2)]
        blobF_t = const.tile([128, 261], F32, tag="blobF", name="blobF_t")
        blobB_t = const.tile([128, 1408], BF16, tag="blobB", name="blobB_t")

        qT_t = [io.tile([128, TQ], F32R, tag=f"qT{c}", name=f"qT{c}")
                for c in range(2)]
        kT_t = [io.tile([128, TQ], F32R, tag=f"kT{c}", name=f"kT{c}")
                for c in range(2)]
        kN_t = io.tile([128, NT * KW], BF16, tag="kN", name="kN_t")
        # 128 pad cols so score block 15 streams a full 256-wide window
        qsT_t = [io.tile([128, TQ + 128], F32R, tag=f"qsT{c}", name=f"qsT{c}")
                 for c in range(2)]
        cT_t = [io.tile([128, TQ], BF16, tag=f"cT{c}", name=f"cT{c}")
                for c in range(2)]
        enhT_t = [io.tile([128, TQ], BF16, tag=f"enhT{c}", name=f"enhT{c}")
                  for c in range(2)]
        oT_sb = [io.tile([128, TQ], BF16, tag=f"oT{c}", name=f"oT{c}")
                 for c in range(2)]
        oT_row = io.tile([1, TQ], BF16, tag="oTr", name="oT_row")

        # const views
        maskC = blobF_t[:, 0:256]              # [diag keep s'<=t' | prev s'>=t']
        beS = [blobF_t[:, 256 + f:257 + f] for f in range(2)]
        bmS = [blobF_t[:, 258 + ci:259 + ci] for ci in range(3)]
        identB = blobB_t[:, 0:128]
        web = [blobB_t[:, 128 + d * 256: 128 + (d + 1) * 256] for d in range(2)]
        wmp = [blobB_t[:, 640 + f * 384: 640 + (f + 1) * 384] for f in range(2)]

        # ---- DMA loads: sync queue in consumption-priority order ----
        nc.sync.dma_start(wst[0][:], WsT[0:128, :])
        nc.sync.dma_start(wst[1][:], WsT[128:256, :])
        nc.sync.dma_start(qT_t[0][:, 0:512], qT[0:128, 0:512])
        nc.sync.dma_start(qT_t[1][:, 0:512], qT[128:256, 0:512])
        nc.sync.dma_start(kT_t[0][:, 0:512], kT[0:128, 0:512])
        nc.sync.dma_start(kT_t[1][:, 0:512], kT[128:256, 0:512])
        nc.sync.dma_start(blobF_t[:], blobF[:])
        nc.sync.dma_start(qT_t[0][:, 512:1024], qT[0:128, 512:1024])
        nc.sync.dma_start(qT_t[1][:, 512:1024], qT[128:256, 512:1024])
        nc.sync.dma_start(blobB_t[:], blobB[:])
        nc.sync.dma_start(kT_t[0][:, 512:1024], kT[0:128, 512:1024])
        nc.sync.dma_start(kT_t[1][:, 512:1024], kT[128:256, 512:1024])
        nc.sync.dma_start(qT_t[0][:, 1024:1536], qT[0:128, 1024:1536])
        nc.sync.dma_start(qT_t[1][:, 1024:1536], qT[128:256, 1024:1536])
        nc.sync.dma_start(weq[0][:], WeTq[0:128, :])
        nc.sync.dma_start(weq[1][:], WeTq[128:256, :])
        nc.sync.dma_start(kT_t[0][:, 1024:1536], kT[0:128, 1024:1536])
        nc.sync.dma_start(kT_t[1][:, 1024:1536], kT[128:256, 1024:1536])
        nc.sync.dma_start(qT_t[0][:, 1536:2048], qT[0:128, 1536:2048])
        nc.sync.dma_start(qT_t[1][:, 1536:2048], qT[128:256, 1536:2048])
        nc.sync.dma_start(kT_t[0][:, 1536:2048], kT[0:128, 1536:2048])
        nc.sync.dma_start(kT_t[1][:, 1536:2048], kT[128:256, 1536:2048])
        # kN pre-rearranged on host -> plain wide loads (8KB descriptors)
        for (b0, b1) in ((0, 8), (8, 16)):
            nc.gpsimd.dma_start(kN_t[:, b0 * KW: b1 * KW],
                                kN[:, b0 * KW: b1 * KW])

        # ---- stage emitters ----
        def emit_p0(nb):
            # qsT[g, t'] = (q @ W_score.T).T : stationary wst, moving qT
            for c in range(2):
                ps = pmm.tile([128, 512], F32, tag="mm", name="ps")
                for h in range(2):
                    nc.tensor.matmul(
                        ps[:],
                        wst[h][:, c * 128:(c + 1) * 128],
                        qT_t[h][:, nb * 512:(nb + 1) * 512],
                        start=(h == 0), stop=(h == 1))
                nc.vector.tensor_copy(qsT_t[c][:, nb * 512:(nb + 1) * 512],
                                      ps[:])

        def emit_sc(m):
            # scoreT[s' of block m, t' of tiles m,m+1] + mask -> exp -> w~T
            # (block 15's upper half reads qsT pad cols: garbage, never used)
            ps = psc.tile([128, 256], F32, tag="sc", name="ps")
            for g in range(2):
                nc.tensor.matmul(
                    ps[:],
                    kT_t[g][:, m * 128:(m + 1) * 128],
                    qsT_t[g][:, m * 128: m * 128 + 256],
                    start=(g == 0), stop=(g == 1))
            sb = wk.tile([128, 256], F32, tag="scb", bufs=3, name="sb")
            nc.vector.tensor_add(sb[:], ps[:], maskC[:])
            wt = wk.tile([128, 256], BF16, tag="wt", bufs=5, name="wt")
            nc.scalar.activation(wt[:], sb[:], AF.Exp)
            return wt

        wT = [None] * NT

        def emit_pv(j):
            # c~[t', 258] = sum_blocks w~T.T @ [kN | 1]
            pc = ppv.tile([128, KW], F32, tag="pv", name="pc")
            if j == 0:
                nc.tensor.matmul(pc[:], wT[0][:, 0:128],
                                 kN_t[:, 0:KW], start=True, stop=True)
            else:
                nc.tensor.matmul(pc[:], wT[j - 1][:, 128:256],
                                 kN_t[:, (j - 1) * KW: j * KW],
                                 start=True, stop=False)
                nc.tensor.matmul(pc[:], wT[j][:, 0:128],
                                 kN_t[:, j * KW: (j + 1) * KW],
                                 start=False, stop=True)
            rec = stat.tile([128, 1], F32, tag="rec", bufs=4, name="rec")
            nc.vector.reciprocal(rec[:], pc[:, 256:257])
            cb = wk.tile([128, 256], BF16, tag="cb", bufs=4, name="cb")
            nc.vector.tensor_scalar_mul(cb[:], pc[:, 0:256], rec[:])
            return cb

        cB = [None] * NT

        def emit_tr(j):
            # cT[h, t'] via PE transpose (bf16), gpsimd copies psum->sbuf
            for h in range(2):
                pt = ptr.tile([128, 128], BF16, tag="tr", name="pt")
                nc.tensor.transpose(pt[:], cB[j][:, h * 128:(h + 1) * 128],
                                    identB)
                # gpsimd cannot read PSUM; split copies across DVE and ACT
                if h == 0:
                    nc.vector.tensor_copy(cT_t[h][:, j * 128:(j + 1) * 128],
                                          pt[:])
                else:
                    nc.scalar.copy(cT_t[h][:, j * 128:(j + 1) * 128], pt[:])

        def emit_p2(g):
            # P2: enhT[f, t'] = tanh(W_enh.T @ [cT;qT] + be)
            for f in range(2):
                pe_ = pmm.tile([128, 512], F32, tag="mm", name="pe_")
                nc.tensor.matmul(pe_[:], web[0][:, f * 128:(f + 1) * 128],
                                 cT_t[0][:, g * 512:(g + 1) * 512],
                                 start=True, stop=False)
                nc.tensor.matmul(pe_[:], web[1][:, f * 128:(f + 1) * 128],
                                 cT_t[1][:, g * 512:(g + 1) * 512],
                                 start=False, stop=False)
                nc.tensor.matmul(pe_[:], weq[0][:, f * 128:(f + 1) * 128],
                                 qT_t[0][:, g * 512:(g + 1) * 512],
                                 start=False, stop=False)
                nc.tensor.matmul(pe_[:], weq[1][:, f * 128:(f + 1) * 128],
                                 qT_t[1][:, g * 512:(g + 1) * 512],
                                 start=False, stop=True)
                nc.scalar.activation(enhT_t[f][:, g * 512:(g + 1) * 512],
                                     pe_[:], AF.Tanh, bias=beS[f])

        def emit_p3(g):
            # P3 transposed: outT[o, t'] = 0.5*tanh(0.5*(Wm@enh + bm)) + 0.5
            for ci in range(3):
                p3 = pmm.tile([128, 512], F32, tag="mm", name="p3")
                for f in range(2):
                    nc.tensor.matmul(
                        p3[:], wmp[f][:, ci * 128:(ci + 1) * 128],
                        enhT_t[f][:, g * 512:(g + 1) * 512],
                        start=(f == 0), stop=(f == 1))
                if ci < 2:
                    os = wk.tile([128, 512], BF16, tag="os", bufs=2, name="os")
                    nc.scalar.activation(os[:], p3[:], AF.Tanh,
                                         scale=0.5, bias=bmS[ci])
                    nc.gpsimd.tensor_scalar(
                        oT_sb[ci][:, g * 512:(g + 1) * 512], os[:],
                        0.5, 0.5, op0=ALU.mult, op1=ALU.add)
                else:
                    os1 = wk.tile([1, 512], BF16, tag="os1", bufs=2,
                                  name="os1")
                    nc.scalar.activation(os1[:], p3[0:1, :], AF.Tanh,
                                         scale=0.5, bias=bmS[2][0:1, :])
                    nc.gpsimd.tensor_scalar(
                        oT_row[0:1, g * 512:(g + 1) * 512], os1[:],
                        0.5, 0.5, op0=ALU.mult, op1=ALU.add)
            emit_stores(g)

        def emit_stores(g):
            c0, c1 = g * 512, (g + 1) * 512
            nc.sync.dma_start(out[0:128, c0:c1], oT_sb[0][:, c0:c1])
            nc.sync.dma_start(out[128:256, c0:c1], oT_sb[1][:, c0:c1])
            nc.sync.dma_start(out[256:257, c0:c1], oT_row[0:1, c0:c1])

        # ---- software-pipelined emission ----
        LPV, LTR = 2, 3     # pv lags scores by 2 steps, transposes by 3

        def emit_lagged(step):
            jpv = step - LPV
            if 0 <= jpv < NT:
                cB[jpv] = emit_pv(jpv)
            jtr = step - LTR
            if 0 <= jtr < NT:
                emit_tr(jtr)
            if step >= 7 and (step - 7) % 4 == 0 and (step - 7) // 4 < 4:
                emit_p2((step - 7) // 4)
            if step >= 8 and (step - 8) % 4 == 0 and (step - 8) // 4 < 4:
                emit_p3((step - 8) // 4)

        m_next = 0
        for nb in range(4):
            emit_p0(nb)
            hi = 4 * nb + 2 if nb < 3 else NT - 1
            while m_next <= hi:
                wT[m_next] = emit_sc(m_next)
                emit_lagged(m_next)
                m_next += 1
        for step in range(NT, NT + 9):
            emit_lagged(step)

    return nc


def _pad_T(x):
    p = np.zeros((TQ, H), np.float32)
    p[:T] = x
    return p


def make_in_maps(k, q, W_score, W_enh, b_enh, W_mask, b_mask):
    k = np.asarray(k, np.float32)
    q = np.asarray(q, np.float32)
    W_score = np.asarray(W_score, np.float32)
    W_enh = np.asarray(W_enh, np.float32)
    b_enh = np.asarray(b_enh, np.float32)
    W_mask = np.asarray(W_mask, np.float32)
    b_mask = np.asarray(b_mask, np.float32)

    WsT = np.ascontiguousarray(W_score.T)                  # [h, g]
    WeT = W_enh.T                                          # [d, f] (512, 256)
    WeTq = np.ascontiguousarray(WeT[H:2 * H])              # q-feature half

    sI = np.arange(128, dtype=np.int32)[:, None]
    tI = np.arange(128, dtype=np.int32)[None, :]
    diag = np.where(sI <= tI, 0.0, NEG).astype(np.float32)
    prev = np.where(sI >= tI, 0.0, NEG).astype(np.float32)
    blobF = np.zeros((128, 261), np.float32)
    blobF[:, 0:128] = diag
    blobF[:, 128:256] = prev
    blobF[:, 256:258] = b_enh.reshape(2, 128).T
    bmh = np.zeros(384, np.float32)
    bmh[:F_OUT] = 0.5 * b_mask
    blobF[:, 258:261] = bmh.reshape(3, 128).T

    blobB = np.zeros((128, 1408), np.float32)
    blobB[:, 0:128] = np.eye(128, dtype=np.float32)
    blobB[:, 128:384] = WeT[0:128]
    blobB[:, 384:640] = WeT[128:256]
    WmP = np.zeros((H, 384), np.float32)
    WmP[:, :F_OUT] = W_mask.T
    blobB[:, 640:1024] = WmP[0:128]
    blobB[:, 1024:1408] = WmP[128:256]
    blobB = blobB.astype(ml_dtypes.bfloat16)

    in_maps = []
    for b in range(N_CORES):
        kb = _pad_T(k[b])
        qb = _pad_T(q[b])
        kNb = np.zeros((TQ, KW), np.float32)
        kNb[:, 0:H] = kb
        kNb[:, 256] = 1.0
        # pre-rearrange into the SBUF layout [p, block*KW + h]
        kNr = np.ascontiguousarray(
            kNb.reshape(NT, 128, KW).transpose(1, 0, 2).reshape(128, NT * KW))
        in_maps.append({
            "qT": np.ascontiguousarray(qb.T),
            "kT": np.ascontiguousarray(kb.T),
            "kN": kNr.astype(ml_dtypes.bfloat16),
            "WsT": WsT, "WeTq": WeTq,
            "blobF": blobF, "blobB": blobB,
        })
    return in_maps


def assemble_output(results):
    outs = []
    for r in results:
        o = np.asarray(r["out"]).astype(np.float32)        # [258, 2048]
        outs.append(np.ascontiguousarray(o[:F_OUT, :T].T))  # [2000, 257]
    return np.stack(outs, 0)


def get_nc():
    if "nc" not in _CACHE:
        nc = build_nc()
        nc.finalize()
        _CACHE["nc"] = nc
    return _CACHE["nc"]


def kernel(k, q, W_score, W_enh, b_enh, W_mask, b_mask):
    in_maps = make_in_maps(k, q, W_score, W_enh, b_enh, W_mask, b_mask)
    res = run_bass_kernel_spmd(get_nc(), in_maps, list(range(N_CORES)))
    return assemble_output(res.results)
